# revision 1
# baseline (speedup 1.0000x reference)
"""DIN attention layer kernel for Trainium2 (8 NeuronCores, data-parallel over batch).

Reference math:
  x  = concat([q, ub, q-ub, q*ub], -1)             # [B,T,144]
  h1 = sigmoid(x @ W1 + b1)                        # [B,T,80]
  h2 = sigmoid(h1 @ W2 + b2)                       # [B,T,40]
  s  = h2 @ W3 + b3                                # [B,T,1]
  w  = softmax(s.T * mask)                         # [B,1,T]  (multiplicative mask)
  out = w @ ub                                     # [B,1,36]

Host-side algebraic folds:
  1) x @ W1 = ub @ (Wb-Wc) + (q*ub) @ Wd + q @ (Wa+Wc); q is per-batch, so fold
     into per-batch weights Waug_b = [(Wb-Wc) + diag(q_b) Wd ; q_b(Wa+Wc)+b1]
     ([37,80]) and augment ub with a ones column -> single K=37 matmul.
  2) sigmoid(x) = 0.5 + 0.5*tanh(x/2); tanh and exp share one ACT table set
     (sigmoid does not), so the device computes t = tanh(pre/2) and the
     0.5/0.5 affine is folded into the next layer's weights/biases.
"""

from contextlib import ExitStack

import numpy as np

import concourse.bass as bass
import concourse.bacc as bacc
import concourse.tile as tile
from concourse import mybir
from concourse.bass_utils import run_bass_kernel_spmd

B, T, E = 4096, 200, 36
N_CORES = 8
F32 = mybir.dt.float32
AF = mybir.ActivationFunctionType


def _segments(start, end, step=128):
    segs = []
    s = start
    while s < end:
        e = min(end, (s // step + 1) * step)
        segs.append((s, e))
        s = e
    return segs


def build_module(bc=512, pb=64, stage=99):
    """bc = batches per core, pb = batches per phase (pb % 32 == 0 keeps
    phases aligned to 128-row tiles, 200-row batches, and 6400-row halves)."""
    assert bc % pb == 0 and pb % 32 == 0 and pb <= 64
    ph_n = bc // pb
    rp = 200 * pb                # rows per phase
    nt = rp // 128               # 128-row tiles per phase
    npair = nt // 2              # transpose pairs per phase
    nhalf = pb // 32             # 32-batch half-phases per phase
    hpair = npair // nhalf       # pairs per half-phase (25)
    sm = pb                      # softmax tile partitions

    nc = bacc.Bacc(
        "TRN2", target_bir_lowering=False, debug=False,
        enable_asserts=False, num_devices=N_CORES,
    )

    ubaug_d = nc.dram_tensor("ubaug", [bc * 200, 37], F32, kind="ExternalInput").ap()
    waug_d = nc.dram_tensor("waug", [bc, 37, 80], F32, kind="ExternalInput").ap()
    lens_d = nc.dram_tensor("lens", [bc, 1], F32, kind="ExternalInput").ap()
    w2_d = nc.dram_tensor("w2", [80, 64], F32, kind="ExternalInput").ap()
    w3d0_d = nc.dram_tensor("w3d0", [104, 32], F32, kind="ExternalInput").ap()
    w3d1_d = nc.dram_tensor("w3d1", [104, 32], F32, kind="ExternalInput").ap()
    b2c_d = nc.dram_tensor("b2c", [128, 1], F32, kind="ExternalInput").ap()
    b3c_d = nc.dram_tensor("b3c", [128, 1], F32, kind="ExternalInput").ap()
    out_d = nc.dram_tensor("out", [bc, 36], F32, kind="ExternalOutput").ap()
    sc_dram = nc.dram_tensor("sc_scratch", [bc * 200], F32, kind="Internal").ap()
    w_dram = nc.dram_tensor("w_scratch", [bc * 200], F32, kind="Internal").ap()

    ident_d = nc.inline_tensor(np.eye(128, dtype=np.float32), name="ident").ap()
    iota_d = nc.inline_tensor(
        np.broadcast_to(np.arange(200, dtype=np.float32), (64, 200)).copy(),
        name="iotat").ap()
    fmA = np.zeros((128, nt), dtype=np.float32)
    fmB = np.zeros((128, nt), dtype=np.float32)
    for k in range(nt):
        b_lo = (128 * k) // 200
        for p in range(128):
            if (128 * k + p) // 200 == b_lo:
                fmA[p, k] = 1.0
            else:
                fmB[p, k] = 1.0
    fmA_d = nc.inline_tensor(fmA, name="fmA").ap()
    fmB_d = nc.inline_tensor(fmB, name="fmB").ap()

    with tile.TileContext(nc) as tc, ExitStack() as es:
        cpool = es.enter_context(tc.tile_pool(name="consts", bufs=1))
        xaugp = es.enter_context(tc.tile_pool(name="xaug", bufs=2))
        h1p = es.enter_context(tc.tile_pool(name="h1p", bufs=4))
        h2p = es.enter_context(tc.tile_pool(name="h2p", bufs=4))
        scbp = es.enter_context(tc.tile_pool(name="scbp", bufs=3))
        smp = es.enter_context(tc.tile_pool(name="smp", bufs=2))
        wcp = es.enter_context(tc.tile_pool(name="wcp", bufs=2))
        outp = es.enter_context(tc.tile_pool(name="outp", bufs=2))
        tpp = es.enter_context(tc.tile_pool(name="tpp", bufs=2, space="PSUM"))
        m1p = es.enter_context(tc.tile_pool(name="m1p", bufs=2, space="PSUM"))
        m23p = es.enter_context(tc.tile_pool(name="m23p", bufs=2, space="PSUM"))

        ident = cpool.tile([128, 128], F32)
        nc.sync.dma_start(out=ident, in_=ident_d)
        iota_t = cpool.tile([64, 200], F32)
        nc.sync.dma_start(out=iota_t, in_=iota_d)
        w2_t = cpool.tile([80, 64], F32)
        nc.sync.dma_start(out=w2_t, in_=w2_d)
        w3_0t = cpool.tile([104, 32], F32)
        nc.sync.dma_start(out=w3_0t, in_=w3d0_d)
        w3_1t = cpool.tile([104, 32], F32)
        nc.sync.dma_start(out=w3_1t, in_=w3d1_d)
        b2_t = cpool.tile([128, 1], F32)
        nc.sync.dma_start(out=b2_t, in_=b2c_d)
        b3_t = cpool.tile([128, 1], F32)
        nc.sync.dma_start(out=b3_t, in_=b3c_d)
        fmA_t = cpool.tile([128, nt], F32)
        nc.sync.dma_start(out=fmA_t, in_=fmA_d)
        fmB_t = cpool.tile([128, nt], F32)
        nc.sync.dma_start(out=fmB_t, in_=fmB_d)
        # zero-padded, manually double-buffered data/weight tensors
        waug0 = cpool.tile([101, 2, 32, 80], F32)
        nc.vector.memset(waug0, 0.0)
        waug1 = cpool.tile([101, 2, 32, 80], F32)
        nc.vector.memset(waug1, 0.0)
        nat2 = cpool.tile([128, 2, npair, 101], F32)
        nc.vector.memset(nat2, 0.0)

        for ph in range(ph_n):
            slot = ph % 2
            # ---- load nat tiles: [128, slot, pair, {0:37 | 64:101}] ----
            for d in range((npair + 9) // 10):
                p0 = 10 * d
                pn = min(10, npair - p0)
                for blk in range(2):
                    nsrc = bass.AP(
                        tensor=ubaug_d.tensor,
                        offset=ubaug_d.offset
                        + 37 * (rp * ph + 256 * p0 + 128 * blk),
                        ap=[[37, 128], [37 * 256, pn], [1, 37]],
                    )
                    nc.sync.dma_start(
                        out=nat2[:, slot, p0:p0 + pn, 64 * blk:64 * blk + 37],
                        in_=nsrc)

            lencol = smp.tile([sm, 1], F32, tag="lencol")
            nc.sync.dma_start(
                out=lencol,
                in_=bass.AP(tensor=lens_d.tensor,
                            offset=lens_d.offset + pb * ph,
                            ap=[[1, sm], [1, 1]]))

            sc_ps = None
            for hp2 in range(nhalf):
                wslot = (ph * nhalf + hp2) % 2
                wsrc = bass.AP(
                    tensor=waug_d.tensor,
                    offset=waug_d.offset + 37 * 80 * (pb * ph + 32 * hp2),
                    ap=[[80, 37], [37 * 80, 32], [1, 80]],
                )
                nc.sync.dma_start(out=waug0[0:37, wslot, :, :], in_=wsrc)
                wsrc2 = bass.AP(
                    tensor=waug_d.tensor,
                    offset=waug_d.offset + 37 * 80 * (pb * ph + 32 * hp2),
                    ap=[[80, 37], [37 * 80, 32], [1, 80]],
                )
                nc.sync.dma_start(out=waug1[64:101, wslot, :, :], in_=wsrc2)

                # ---- transpose this half-phase into xaug [101, hpair*128] ----
                xaug = xaugp.tile([101, hpair * 128], F32, tag="xaug")
                ngrp = (hpair + 3) // 4
                for g in range(ngrp):
                    tp_ps = tpp.tile([101, 512], F32, tag="tps")
                    pg0 = 4 * g
                    pgn = min(4, hpair - pg0)
                    for j in range(pgn):
                        nc.tensor.transpose(
                            tp_ps[0:101, 128 * j:128 * (j + 1)],
                            nat2[:, slot, hpair * hp2 + pg0 + j, :], ident)
                    nc.vector.tensor_copy(
                        out=xaug[0:101, 512 * g:512 * g + 128 * pgn],
                        in_=tp_ps[0:101, 0:128 * pgn])

                if stage <= 1:
                    dbg = h1p.tile([101, 512], F32, tag="dbg1")
                    nc.vector.tensor_copy(out=dbg, in_=xaug[:, 0:512])
                    nc.sync.dma_start(out=out_d[0:32, :], in_=dbg[0:32, 0:36])
                    continue

                # ---- MLP: 4 batches per mm1 psum tile ----
                for q4 in range(8):
                    m1_ps = m1p.tile([80, 1024], F32, tag="m1")
                    for j in range(4):
                        bl32 = 4 * q4 + j          # batch within half-phase
                        bl = 32 * hp2 + bl32       # batch within phase
                        colbase = 512 * (j // 2) + 200 * (j % 2)
                        for (rs, re) in _segments(200 * bl32, 200 * bl32 + 200):
                            k = rs // 128
                            i0 = rs % 128
                            c0 = 128 * (k // 2) + i0
                            wt_sel = waug0 if k % 2 == 0 else waug1
                            nc.tensor.matmul(
                                m1_ps[0:80, colbase + rs - 200 * bl32:
                                      colbase + re - 200 * bl32],
                                wt_sel[0:101, wslot, bl32, :],
                                xaug[0:101, c0:c0 + (re - rs)],
                                start=True, stop=True)
                    if stage == 15:
                        if q4 == 0:
                            dbg2 = h1p.tile([80, 400], F32, tag="dbg2")
                            nc.vector.tensor_copy(out=dbg2,
                                                  in_=m1_ps[0:80, 0:400])
                            nc.sync.dma_start(out=out_d[0:32, :],
                                              in_=dbg2[0:32, 0:36])
                        continue
                    h1_t = h1p.tile([80, 2, 400], F32, tag="h1")
                    nc.scalar.activation(
                        out=h1_t,
                        in_=m1_ps.rearrange("p (u c) -> p u c", u=2)[:, :, 0:400],
                        func=AF.Tanh, scale=0.5)
                    if stage <= 2:
                        if q4 == 0:
                            nc.sync.dma_start(out=out_d[0:32, :],
                                              in_=h1_t[0:32, 0, 0:36])
                        continue
                    m2_ps = m23p.tile([128, 512], F32, tag="m23")
                    for j in range(4):
                        ps = 64 * (j % 2)
                        ch = 200 * (j // 2)
                        nc.tensor.matmul(
                            m2_ps[ps:ps + 64, ch:ch + 200], w2_t,
                            h1_t[:, j // 2, (j % 2) * 200:(j % 2) * 200 + 200],
                            start=True, stop=True)
                    h2_t = h2p.tile([128, 400], F32, tag="h2")
                    nc.scalar.activation(
                        out=h2_t, in_=m2_ps[0:128, 0:400],
                        func=AF.Tanh, bias=b2_t, scale=0.5)
                    if stage <= 3:
                        if q4 == 0:
                            nc.sync.dma_start(out=out_d[0:32, :],
                                              in_=h2_t[0:32, 0:36])
                        continue
                    if q4 % 2 == 0:
                        sc_ps = m23p.tile([128, 400], F32, tag="m23")
                    for j in range(4):
                        bl8 = (4 * q4 + j) % 8
                        pslot = 32 * (bl8 // 2)
                        half = 200 * (bl8 % 2)
                        ps = 64 * (j % 2)
                        ch = 200 * (j // 2)
                        w3_sel = w3_0t if ps == 0 else w3_1t
                        nc.tensor.matmul(
                            sc_ps[pslot:pslot + 32, half:half + 200],
                            w3_sel, h2_t[0:104, ch:ch + 200],
                            start=True, stop=True, tile_position=(0, pslot))
                    if q4 % 2 == 1:
                        scb = scbp.tile([97, 400], F32, tag="scb")
                        nc.vector.tensor_copy(out=scb, in_=sc_ps[0:97, :])
                        g8 = (32 * hp2) // 8 + q4 // 2
                        nc.sync.dma_start(
                            out=bass.AP(
                                tensor=sc_dram.tensor,
                                offset=sc_dram.offset
                                + 200 * (pb * ph + 8 * g8),
                                ap=[[400, 4], [200, 2], [1, 200]]),
                            in_=bass.AP(
                                tensor=scb.tensor, offset=scb.offset,
                                ap=[[400 * 32, 4], [200, 2], [1, 200]]))

            if stage <= 4:
                continue

            # ---- masked softmax over t, batches on partitions ----
            sc_t = smp.tile([sm, 200], F32, tag="sc")
            nc.sync.dma_start(
                out=sc_t,
                in_=bass.AP(tensor=sc_dram.tensor,
                            offset=sc_dram.offset + 200 * pb * ph,
                            ap=[[200, sm], [1, 200]]))
            mask_t = smp.tile([sm, 200], F32, tag="mask")
            nc.vector.tensor_scalar(
                out=mask_t, in0=iota_t[0:sm, :], scalar1=lencol, scalar2=None,
                op0=mybir.AluOpType.is_lt)
            sb3 = smp.tile([sm, 200], F32, tag="sb3")
            nc.vector.tensor_scalar_add(sb3, sc_t, b3_t[0:sm, :])
            masked = smp.tile([sm, 200], F32, tag="masked")
            nc.vector.tensor_mul(masked, sb3, mask_t)
            negmax = smp.tile([sm, 1], F32, tag="negmax")
            nc.vector.tensor_reduce(
                out=negmax, in_=masked, axis=mybir.AxisListType.X,
                op=mybir.AluOpType.max, negate=True)
            ew = smp.tile([sm, 200], F32, tag="ew")
            sumexp = smp.tile([sm, 1], F32, tag="sumexp")
            nc.scalar.activation(
                out=ew, in_=masked, func=AF.Exp,
                bias=negmax, accum_out=sumexp)
            rz = smp.tile([sm, 1], F32, tag="rz")
            nc.vector.reciprocal(rz, sumexp)
            w_t = smp.tile([sm, 200], F32, tag="wt")
            nc.vector.tensor_scalar_mul(w_t, ew, rz)
            nc.sync.dma_start(
                out=bass.AP(tensor=w_dram.tensor,
                            offset=w_dram.offset + rp * ph,
                            ap=[[200, sm], [1, 200]]),
                in_=w_t)

            if stage <= 5:
                continue

            # ---- weighted sum of ub rows (softmax @ ub) ----
            wcols = wcp.tile([128, nt], F32, tag="wcols")
            nc.sync.dma_start(
                out=wcols,
                in_=bass.AP(tensor=w_dram.tensor,
                            offset=w_dram.offset + rp * ph,
                            ap=[[1, 128], [128, nt]]))
            wA = wcp.tile([128, nt], F32, tag="wA")
            nc.vector.tensor_mul(wA, wcols, fmA_t)
            wB = wcp.tile([128, nt], F32, tag="wB")
            nc.vector.tensor_mul(wB, wcols, fmB_t)

            if stage <= 6:
                continue

            n_mv = (pb + 51) // 52
            mv_tiles = []
            for _i in range(n_mv):
                mv_t = m23p.tile([97, 481], F32, tag="m23", name=f"mv{_i}")
                nc.vector.memset(mv_t[0:97, 0:481], 0.0)
                mv_tiles.append(mv_t)

            def emit_mv(bb, w_tile, k, rhs):
                t0 = (200 * bb) // 128
                t1 = (200 * bb + 199) // 128
                mv = mv_tiles[bb // 52]
                bi = bb % 52
                pslot = 32 * (bi % 4)
                colr = 37 * (bi // 4)
                nc.tensor.matmul(
                    mv[pslot:pslot + 1, colr:colr + 37],
                    w_tile[:, k:k + 1], rhs,
                    start=(k == t0), stop=(k == t1),
                    tile_position=(0, pslot))

            for k in range(nt):
                b_lo = (128 * k) // 200
                b_hi = (128 * k + 127) // 200
                rhs = nat2[:, slot, k // 2, 64 * (k % 2):64 * (k % 2) + 37]
                if b_lo == b_hi:
                    emit_mv(b_lo, wcols, k, rhs)
                else:
                    emit_mv(b_lo, wA, k, rhs)
                    emit_mv(b_hi, wB, k, rhs)

            for i in range(n_mv):
                nb = min(52, pb - 52 * i)
                ncolr = (nb + 3) // 4
                o_t = outp.tile([97, 481], F32, tag="out")
                nc.vector.tensor_copy(
                    out=o_t[0:97, 0:37 * ncolr],
                    in_=mv_tiles[i][0:97, 0:37 * ncolr])
                nc.sync.dma_start(
                    out=bass.AP(
                        tensor=out_d.tensor,
                        offset=out_d.offset + 36 * (pb * ph + 52 * i),
                        ap=[[36, min(4, nb)], [144, ncolr], [1, 36]]),
                    in_=bass.AP(
                        tensor=o_t.tensor, offset=o_t.offset,
                        ap=[[481 * 32, min(4, nb)], [37, ncolr], [1, 36]]))

    nc.compile()
    return nc


def host_prep(query_ad, user_behavior, user_behavior_length,
              W1, b1, W2, b2, W3, b3, bc):
    q = np.asarray(query_ad, dtype=np.float32)
    ub = np.asarray(user_behavior, dtype=np.float32)
    lens = np.asarray(user_behavior_length)
    W1 = np.asarray(W1, dtype=np.float32)
    b1 = np.asarray(b1, dtype=np.float32)
    W2 = np.asarray(W2, dtype=np.float32)
    b2 = np.asarray(b2, dtype=np.float32)
    W3 = np.asarray(W3, dtype=np.float32)
    b3 = np.asarray(b3, dtype=np.float32)
    nb = q.shape[0]

    Wa, Wb, Wc, Wd = W1[0:36], W1[36:72], W1[72:108], W1[108:144]
    waug = np.empty((nb, 37, 80), dtype=np.float32)
    waug[:, 0:36, :] = (Wb - Wc)[None, :, :] + q[:, :, None] * Wd[None, :, :]
    waug[:, 36, :] = q @ (Wa + Wc) + b1[None, :]

    ubaug = np.empty((nb, 200, 37), dtype=np.float32)
    ubaug[:, :, 0:36] = ub
    ubaug[:, :, 36] = 1.0

    # sigmoid -> tanh fold: h = 0.5 + 0.5*t with t = tanh(pre/2)
    w2f = 0.5 * W2                                   # device mm2 weights
    b2f = 0.5 * (b2 + 0.5 * W2.sum(axis=0))          # ACT bias (scale=0.5 applied)
    w3f = 0.5 * W3
    b3f = float(b3[0] + 0.5 * W3.sum())

    w2p = np.zeros((80, 64), dtype=np.float32)
    w2p[:, 0:40] = w2f
    w3d0 = np.zeros((104, 32), dtype=np.float32)
    w3d0[0:40, 0] = w3f[:, 0]
    w3d1 = np.zeros((104, 32), dtype=np.float32)
    w3d1[64:104, 0] = w3f[:, 0]
    b2c = np.zeros((128, 1), dtype=np.float32)
    b2c[0:40, 0] = b2f
    b2c[64:104, 0] = b2f
    b3c = np.full((128, 1), b3f, dtype=np.float32)

    n_cores = nb // bc
    in_maps = []
    for c in range(n_cores):
        sl = slice(bc * c, bc * (c + 1))
        in_maps.append({
            "ubaug": np.ascontiguousarray(ubaug[sl].reshape(bc * 200, 37)),
            "waug": np.ascontiguousarray(waug[sl]),
            "lens": lens[sl].astype(np.float32).reshape(bc, 1),
            "w2": w2p, "w3d0": w3d0, "w3d1": w3d1, "b2c": b2c, "b3c": b3c,
        })
    return in_maps


_NC_CACHE = {}


def get_module(bc, pb):
    key = (bc, pb)
    if key not in _NC_CACHE:
        _NC_CACHE[key] = build_module(bc, pb)
    return _NC_CACHE[key]


def kernel(query_ad, user_behavior, user_behavior_length,
           W1, b1, W2, b2, W3, b3, trace=False):
    bc = B // N_CORES
    nc = get_module(bc, 64)
    in_maps = host_prep(query_ad, user_behavior, user_behavior_length,
                        W1, b1, W2, b2, W3, b3, bc)
    res = run_bass_kernel_spmd(nc, in_maps, core_ids=list(range(N_CORES)),
                               trace=trace)
    outs = [res.results[c]["out"] for c in range(N_CORES)]
    full = np.concatenate(outs, axis=0).reshape(B, 1, 36)
    if trace:
        kernel.last_result = res
    return full



# revision 24
# speedup vs baseline: 2.0604x; 2.0604x over previous
"""DIN attention layer kernel for Trainium2 (8 NeuronCores, data-parallel over batch).

Reference math:
  x  = concat([q, ub, q-ub, q*ub], -1)             # [B,T,144]
  h1 = sigmoid(x @ W1 + b1)                        # [B,T,80]
  h2 = sigmoid(h1 @ W2 + b2)                       # [B,T,40]
  s  = h2 @ W3 + b3                                # [B,T,1]
  w  = softmax(s.T * mask)                         # [B,1,T]  (multiplicative mask)
  out = w @ ub                                     # [B,1,36]

Host-side algebraic folds:
  1) x @ W1 = ub @ (Wb-Wc) + (q*ub) @ Wd + q @ (Wa+Wc); q is per-batch, so fold
     into per-batch weights Waug_b = [(Wb-Wc) + diag(q_b) Wd ; q_b(Wa+Wc)+b1]
     ([37,80]) and augment ub with a ones column -> single K=37 matmul.
  2) sigmoid(x) = 0.5 + 0.5*tanh(x/2); tanh and exp share one ACT table set,
     so the device computes t = tanh(pre/2) and the 0.5/0.5 affine is folded
     into the next layer's weights/biases.

Device strategy (v1, bf16):
  - Host pre-packs all DRAM arrays in the exact SBUF layouts so every DMA is
    a few large contiguous descriptors (the fp32 version spent ~775us of SP
    sequencer time generating gather descriptors and saturated DMA ring 0).
  - ubaug is shipped twice: transposed ([37, rows] per phase) feeding mm1
    directly (no on-chip transposes), and batch-aligned natural tiles
    ([128, 2, 37] per batch) feeding the final weighted sum.
  - All matmuls run in bf16 (psum fp32): mm1 streams 200 cols/batch, mm2
    400 cols per 4 batches, mm3 computes 4 batches in one 400-col stream via
    a [104,2] two-block W3, the weighted sum is data-stationary (ub tile as
    lhsT, softmax-weight columns as rhs) at 2 matmuls/batch.
  - Softmax weights are transposed on-chip (PE) instead of a DRAM roundtrip.
  - Work is software-pipelined so PE/ACT/DVE overlap; phase tail work (softmax,
    weighted sum of phase p) is interleaved into phase p+1's MLP groups.
"""

from contextlib import ExitStack

import numpy as np
import ml_dtypes

import concourse.bass as bass
import concourse.bacc as bacc
import concourse.tile as tile
from concourse import mybir
from concourse.bass_utils import run_bass_kernel_spmd

DEBUG_TAPS = False

B, T, E = 4096, 200, 36
N_CORES = 8
BC = B // N_CORES          # batches per core (512)
PB = 64                    # batches per phase
PH = BC // PB              # phases (8)
RP = PB * T                # rows per phase (12800)
F32 = mybir.dt.float32
BF16 = mybir.dt.bfloat16
AF = mybir.ActivationFunctionType
ALU = mybir.AluOpType
BF_NP = ml_dtypes.bfloat16


def dap(t, offset, dims):
    return bass.AP(tensor=t.tensor, offset=t.offset + offset, ap=dims)


def build_module():
    nc = bacc.Bacc(
        "TRN2", target_bir_lowering=False, debug=False,
        enable_asserts=False, num_devices=N_CORES,
    )

    # host-prepacked inputs (layouts match SBUF tiles; all DMAs are contiguous)
    ubt_d = nc.dram_tensor("ubt", [PH, 37, RP], BF16, kind="ExternalInput").ap()
    natb_d = nc.dram_tensor("natb", [PH, 128, PB * 2 * 37], BF16,
                            kind="ExternalInput").ap()
    waugt_d = nc.dram_tensor("waugt", [PH, 37, PB * 80], BF16,
                             kind="ExternalInput").ap()
    w2p_d = nc.dram_tensor("w2p", [80, 64], BF16, kind="ExternalInput").ap()
    w3p_d = nc.dram_tensor("w3p", [104, 2], BF16, kind="ExternalInput").ap()
    b2c_d = nc.dram_tensor("b2c", [128, 1], F32, kind="ExternalInput").ap()
    b3c_d = nc.dram_tensor("b3c", [64, 1], F32, kind="ExternalInput").ap()
    lens_d = nc.dram_tensor("lens", [PH, 64], F32, kind="ExternalInput").ap()
    out_d = nc.dram_tensor("out", [BC, 36], F32, kind="ExternalOutput").ap()
    if DEBUG_TAPS:
        sc_dram = nc.dram_tensor("sc_scratch", [BC * T], F32,
                                 kind="ExternalOutput").ap()
        w_dbg = nc.dram_tensor("w_dbg", [BC, T], F32, kind="ExternalOutput").ap()
        h1_dbg = nc.dram_tensor("h1_dbg", [80, 800], BF16,
                                kind="ExternalOutput").ap()
        h2_dbg = nc.dram_tensor("h2_dbg", [128, 400], BF16,
                                kind="ExternalOutput").ap()
        mv_dbg = nc.dram_tensor("mv_dbg", [37, 64], F32,
                                kind="ExternalOutput").ap()

    ident64_d = nc.inline_tensor(np.eye(64, dtype=np.float32), name="ident64").ap()
    identf_d = nc.inline_tensor(np.eye(37, dtype=np.float32), name="identf").ap()
    iota_d = nc.inline_tensor(
        np.broadcast_to(np.arange(T, dtype=np.float32), (64, T)).copy(),
        name="iotat").ap()

    with tile.TileContext(nc) as tc, ExitStack() as es:
        cpool = es.enter_context(tc.tile_pool(name="consts", bufs=1))
        ubtp = es.enter_context(tc.tile_pool(name="ubtp", bufs=2))
        natp = es.enter_context(tc.tile_pool(name="natp", bufs=3))
        waugp = es.enter_context(tc.tile_pool(name="waugp", bufs=2))
        lensp = es.enter_context(tc.tile_pool(name="lensp", bufs=3))
        h1p = es.enter_context(tc.tile_pool(name="h1p", bufs=3))
        h2p = es.enter_context(tc.tile_pool(name="h2p", bufs=3))
        scbp = es.enter_context(tc.tile_pool(name="scbp", bufs=2))
        smp = es.enter_context(tc.tile_pool(name="smp", bufs=2))
        wtp = es.enter_context(tc.tile_pool(name="wtp", bufs=2))
        mvsp = es.enter_context(tc.tile_pool(name="mvsp", bufs=2))
        otp = es.enter_context(tc.tile_pool(name="otp", bufs=2))
        m1p = es.enter_context(tc.tile_pool(name="m1p", bufs=2, space="PSUM"))
        m2p = es.enter_context(tc.tile_pool(name="m2p", bufs=2, space="PSUM"))
        scp = es.enter_context(tc.tile_pool(name="scp", bufs=1, space="PSUM"))
        smps = es.enter_context(tc.tile_pool(name="smps", bufs=1, space="PSUM"))

        ident64 = cpool.tile([64, 64], F32)
        nc.sync.dma_start(out=ident64, in_=ident64_d)
        identf = cpool.tile([37, 37], F32)
        nc.sync.dma_start(out=identf, in_=identf_d)
        iota_t = cpool.tile([64, T], F32)
        nc.sync.dma_start(out=iota_t, in_=iota_d)
        w2_t = cpool.tile([80, 64], BF16)
        nc.sync.dma_start(out=w2_t, in_=w2p_d)
        w3_t = cpool.tile([104, 2], BF16)
        nc.sync.dma_start(out=w3_t, in_=w3p_d)
        b2_t = cpool.tile([128, 1], F32)
        nc.sync.dma_start(out=b2_t, in_=b2c_d)
        b3_t = cpool.tile([64, 1], F32)
        nc.sync.dma_start(out=b3_t, in_=b3c_d)

        loaded = {}

        def emit_loads(ph):
            ubt_t = ubtp.tile([37, RP], BF16, tag="ubt", name=f"ubt{ph}")
            nc.sync.dma_start(
                out=ubt_t, in_=dap(ubt_d, ph * 37 * RP, [[RP, 37], [1, RP]]))
            nat_t = natp.tile([128, PB, 2, 37], BF16, tag="nat", name=f"nat{ph}")
            nc.sync.dma_start(
                out=nat_t,
                in_=dap(natb_d, ph * 128 * PB * 2 * 37,
                        [[PB * 2 * 37, 128], [1, PB * 2 * 37]]))
            waug_t = waugp.tile([37, PB, 80], BF16, tag="waug", name=f"waug{ph}")
            nc.sync.dma_start(
                out=waug_t,
                in_=dap(waugt_d, ph * 37 * PB * 80,
                        [[PB * 80, 37], [1, PB * 80]]))
            lens_t = lensp.tile([64, 1], F32, tag="lens", name=f"lens{ph}")
            nc.sync.dma_start(
                out=lens_t, in_=dap(lens_d, 64 * ph, [[1, 64], [1, 1]]))
            loaded[ph] = (ubt_t, nat_t, waug_t, lens_t)

        def emit_wt_transposes(ph):
            """Transpose softmax weights of phase ph for the weighted sum."""
            wb = loaded[ph + 100]  # wb tile stored under key ph+100
            smt = smps.tile([128, 128], F32, tag="sm", name=f"smt{ph}")
            nc.tensor.transpose(smt[0:128, 0:64], wb[:, 0:128], ident64)
            nc.tensor.transpose(smt[0:72, 64:128], wb[:, 128:200], ident64)
            wT0 = wtp.tile([128, 64], BF16, tag="wt0", name=f"wt0{ph}")
            nc.vector.tensor_copy(out=wT0, in_=smt[0:128, 0:64])
            wT1 = wtp.tile([72, 64], BF16, tag="wt1", name=f"wt1{ph}")
            nc.vector.tensor_copy(out=wT1, in_=smt[0:72, 64:128])
            loaded[ph + 200] = (smt, wT0, wT1)

        def emit_mv(ph, b0, b1):
            """Weighted-sum matmuls for batches [b0, b1) of phase ph."""
            nat_t = loaded[ph][1]
            smt, wT0, wT1 = loaded[ph + 200]
            for b in range(b0, b1):
                nc.tensor.matmul(
                    smt[0:37, b:b + 1], nat_t[:, b, 0, :], wT0[:, b:b + 1],
                    start=True, stop=False)
                nc.tensor.matmul(
                    smt[0:37, b:b + 1], nat_t[0:72, b, 1, :], wT1[:, b:b + 1],
                    start=False, stop=True)

        def emit_out(ph):
            smt, _, _ = loaded[ph + 200]
            mvs = mvsp.tile([37, 64], F32, tag="mvs", name=f"mvs{ph}")
            nc.vector.tensor_copy(out=mvs, in_=smt[0:37, 0:64])
            if DEBUG_TAPS and ph == 0:
                nc.sync.dma_start(out=mv_dbg, in_=mvs)
            nc.tensor.transpose(smt[0:64, 64:101], mvs, identf)
            ot = otp.tile([64, 37], F32, tag="ot", name=f"ot{ph}")
            nc.vector.tensor_copy(out=ot, in_=smt[0:64, 64:101])
            nc.sync.dma_start(
                out=dap(out_d, 36 * PB * ph, [[36, 64], [1, 36]]),
                in_=dap(ot, 0, [[37, 64], [1, 36]]))

        def emit_softmax(ph, sc_t):
            lens_t = loaded[ph][3]
            if DEBUG_TAPS:
                nc.sync.dma_start(
                    out=dap(sc_dram, T * PB * ph, [[T, 64], [1, T]]), in_=sc_t)
            mask_t = smp.tile([64, T], F32, tag="mask", name=f"mask{ph}")
            nc.vector.tensor_scalar(
                out=mask_t, in0=iota_t, scalar1=lens_t, scalar2=None,
                op0=ALU.is_lt)
            masked = smp.tile([64, T], F32, tag="masked", name=f"masked{ph}")
            nc.vector.scalar_tensor_tensor(
                out=masked, in0=sc_t, scalar=b3_t, in1=mask_t,
                op0=ALU.add, op1=ALU.mult)
            negmax = smp.tile([64, 1], F32, tag="negmax", name=f"negmax{ph}")
            nc.vector.tensor_reduce(
                out=negmax, in_=masked, axis=mybir.AxisListType.X,
                op=ALU.max, negate=True)
            ew = smp.tile([64, T], F32, tag="ew", name=f"ew{ph}")
            sumexp = smp.tile([64, 1], F32, tag="sumexp", name=f"sumexp{ph}")
            nc.scalar.activation(
                out=ew, in_=masked, func=AF.Exp, bias=negmax, accum_out=sumexp)
            rz = smp.tile([64, 1], F32, tag="rz", name=f"rz{ph}")
            nc.vector.reciprocal(rz, sumexp)
            w_t = smp.tile([64, T], F32, tag="wt", name=f"wt{ph}")
            nc.vector.tensor_scalar_mul(w_t, ew, rz)
            if DEBUG_TAPS:
                nc.sync.dma_start(
                    out=dap(w_dbg, T * PB * ph, [[T, 64], [1, T]]), in_=w_t)
            loaded[ph + 100] = w_t

        emit_loads(0)
        for ph in range(PH):
            if ph + 1 < PH:
                emit_loads(ph + 1)
            ubt_t, nat_t, waug_t, lens_t = loaded[ph]
            prev = ph - 1 if ph > 0 else None

            m1_tiles = {}
            h1_tiles = {}
            h2_tiles = {}
            sc_tile = [None]
            sc_t_phase = [smp.tile([64, T], F32, tag="sct", name=f"sct{ph}")]

            # 16 groups of 4 batches + 2 drain iterations, software-pipelined:
            # PE order per iter: mm1(g), [tail work of prev phase], mm2(g-1),
            # mm3(g-2).  ACT order: h1(g), h2(g-1).
            for g in range(18):
                if g < 16:
                    m1_ps = m1p.tile([80, 1024], F32, tag="m1", name=f"m1_{ph}_{g}")
                    for j in range(4):
                        b = 4 * g + j
                        colbase = 512 * (j // 2) + 200 * (j % 2)
                        nc.tensor.matmul(
                            m1_ps[0:80, colbase:colbase + 200],
                            waug_t[:, b, :],
                            ubt_t[:, 200 * b:200 * b + 200],
                            start=True, stop=True)
                    h1_t = h1p.tile([80, 2, 400], BF16, tag="h1", name=f"h1_{ph}_{g}")
                    nc.scalar.activation(
                        out=h1_t,
                        in_=m1_ps.rearrange("p (u c) -> p u c", u=2)[:, :, 0:400],
                        func=AF.Tanh, scale=0.5)
                    m1_tiles[g] = m1_ps
                    h1_tiles[g] = h1_t
                    if DEBUG_TAPS and ph == 0 and g == 0:
                        nc.sync.dma_start(
                            out=dap(h1_dbg, 0, [[800, 80], [1, 800]]),
                            in_=dap(h1_t, 0, [[800, 80], [1, 800]]))

                if prev is not None:
                    if g == 2:
                        emit_wt_transposes(prev)
                    if 3 <= g <= 15:
                        b0 = 5 * (g - 3)
                        b1 = min(64, 5 * (g - 2))
                        emit_mv(prev, b0, b1)
                    if g == 16:
                        emit_out(prev)

                if 1 <= g <= 16:
                    g1 = g - 1
                    h1_t = h1_tiles.pop(g1)
                    m2_ps = m2p.tile([128, 400], F32, tag="m2", name=f"m2_{ph}_{g1}")
                    for u in range(2):
                        nc.tensor.matmul(
                            m2_ps[64 * u:64 * u + 64, 0:400], w2_t,
                            h1_t[:, u, :], start=True, stop=True)
                    h2_t = h2p.tile([128, 400], BF16, tag="h2", name=f"h2_{ph}_{g1}")
                    nc.scalar.activation(
                        out=h2_t, in_=m2_ps, func=AF.Tanh, bias=b2_t, scale=0.5)
                    h2_tiles[g1] = h2_t
                    m1_tiles.pop(g1, None)
                    if DEBUG_TAPS and ph == 0 and g1 == 0:
                        nc.sync.dma_start(
                            out=dap(h2_dbg, 0, [[400, 128], [1, 400]]),
                            in_=dap(h2_t, 0, [[400, 128], [1, 400]]))

                if 2 <= g <= 17:
                    g2 = g - 2
                    q = g2 % 4
                    if q == 0:
                        sc_tile[0] = scp.tile([98, 400], F32, tag="sc",
                                              name=f"sc_{ph}_{g2 // 4}")
                    h2_t = h2_tiles.pop(g2)
                    # batch 4q+2*blk+hc -> psum partition 32q+blk, col half hc
                    nc.tensor.matmul(
                        sc_tile[0][32 * q:32 * q + 2, 0:400], w3_t,
                        h2_t[0:104, 0:400], start=True, stop=True,
                        tile_position=(0, 32 * q))
                    if q == 3:
                        g16 = g2 // 4
                        scb = scbp.tile([98, 400], F32, tag="scb",
                                        name=f"scb_{ph}_{g16}")
                        nc.vector.tensor_copy(out=scb, in_=sc_tile[0])
                        # SBUF->SBUF DMAs compact the partition-paired psum
                        # layout into batch-major rows of sc_t (tracked by
                        # the tile framework; offsets encode partitions as
                        # offset//pitch, strides likewise)
                        for blk in range(2):
                            for hc in range(2):
                                nc.sync.dma_start(
                                    out=dap(sc_t_phase[0],
                                            T * (16 * g16 + 2 * blk + hc),
                                            [[800, 4], [1, 200]]),
                                    in_=dap(scb, 400 * blk + 200 * hc,
                                            [[400 * 32, 4], [1, 200]]))

            emit_softmax(ph, sc_t_phase[0])

        # tail: softmax-weighted sum for the last phase
        emit_wt_transposes(PH - 1)
        emit_mv(PH - 1, 0, 64)
        emit_out(PH - 1)

    nc.compile()
    return nc


def host_prep(query_ad, user_behavior, user_behavior_length,
              W1, b1, W2, b2, W3, b3):
    q = np.asarray(query_ad, dtype=np.float32)
    ub = np.asarray(user_behavior, dtype=np.float32)
    lens = np.asarray(user_behavior_length)
    W1 = np.asarray(W1, dtype=np.float32)
    b1 = np.asarray(b1, dtype=np.float32)
    W2 = np.asarray(W2, dtype=np.float32)
    b2 = np.asarray(b2, dtype=np.float32)
    W3 = np.asarray(W3, dtype=np.float32)
    b3 = np.asarray(b3, dtype=np.float32)
    nb = q.shape[0]

    Wa, Wb, Wc, Wd = W1[0:36], W1[36:72], W1[72:108], W1[108:144]
    waug = np.empty((nb, 37, 80), dtype=np.float32)
    waug[:, 0:36, :] = (Wb - Wc)[None, :, :] + q[:, :, None] * Wd[None, :, :]
    waug[:, 36, :] = q @ (Wa + Wc) + b1[None, :]

    ubaug = np.empty((nb, T, 37), dtype=np.float32)
    ubaug[:, :, 0:36] = ub
    ubaug[:, :, 36] = 1.0

    # sigmoid -> tanh fold: h = 0.5 + 0.5*t with t = tanh(pre/2)
    w2f = 0.5 * W2
    b2f = 0.5 * (b2 + 0.5 * W2.sum(axis=0))
    w3f = 0.5 * W3
    b3f = float(b3[0] + 0.5 * W3.sum())

    w2p = np.zeros((80, 64), dtype=np.float32)
    w2p[:, 0:40] = w2f
    w3p = np.zeros((104, 2), dtype=np.float32)
    w3p[0:40, 0] = w3f[:, 0]
    w3p[64:104, 1] = w3f[:, 0]
    b2c = np.zeros((128, 1), dtype=np.float32)
    b2c[0:40, 0] = b2f
    b2c[64:104, 0] = b2f
    b3c = np.full((64, 1), b3f, dtype=np.float32)

    w2p = w2p.astype(BF_NP)
    w3p = w3p.astype(BF_NP)

    n_cores = nb // BC
    in_maps = []
    for c in range(n_cores):
        sl = slice(BC * c, BC * (c + 1))
        ub_c = ubaug[sl]                                    # [512, 200, 37]
        # mm1 rhs: transposed rows per phase [PH, 37, RP]
        ubt = np.ascontiguousarray(
            ub_c.reshape(PH, RP, 37).transpose(0, 2, 1)).astype(BF_NP)
        # weighted-sum lhsT: batch-aligned natural tiles [PH, 128, PB, 2, 37]
        pad = np.zeros((BC, 256, 37), dtype=np.float32)
        pad[:, 0:T] = ub_c
        natb = np.ascontiguousarray(
            pad.reshape(PH, PB, 2, 128, 37).transpose(0, 3, 1, 2, 4)
        ).reshape(PH, 128, PB * 2 * 37).astype(BF_NP)
        waugt = np.ascontiguousarray(
            waug[sl].reshape(PH, PB, 37, 80).transpose(0, 2, 1, 3)
        ).reshape(PH, 37, PB * 80).astype(BF_NP)
        lensc = lens[sl].astype(np.float32).reshape(PH, PB)
        in_maps.append({
            "ubt": ubt, "natb": natb, "waugt": waugt,
            "w2p": w2p, "w3p": w3p, "b2c": b2c, "b3c": b3c,
            "lens": lensc,
        })
    return in_maps


_NC_CACHE = {}


def get_module():
    if "nc" not in _NC_CACHE:
        _NC_CACHE["nc"] = build_module()
    return _NC_CACHE["nc"]


def kernel(query_ad, user_behavior, user_behavior_length,
           W1, b1, W2, b2, W3, b3, trace=False):
    nc = get_module()
    in_maps = host_prep(query_ad, user_behavior, user_behavior_length,
                        W1, b1, W2, b2, W3, b3)
    res = run_bass_kernel_spmd(nc, in_maps, core_ids=list(range(N_CORES)),
                               trace=trace)
    outs = [res.results[c]["out"] for c in range(N_CORES)]
    full = np.concatenate(outs, axis=0).reshape(B, 1, 36)
    kernel.last_results = res.results
    if trace:
        kernel.last_result = res
    return full


# revision 41
# speedup vs baseline: 2.1865x; 1.0612x over previous
"""DIN attention layer kernel for Trainium2 (8 NeuronCores, data-parallel over batch).

Reference math:
  x  = concat([q, ub, q-ub, q*ub], -1)             # [B,T,144]
  h1 = sigmoid(x @ W1 + b1)                        # [B,T,80]
  h2 = sigmoid(h1 @ W2 + b2)                       # [B,T,40]
  s  = h2 @ W3 + b3                                # [B,T,1]
  w  = softmax(s.T * mask)                         # [B,1,T]  (multiplicative mask)
  out = w @ ub                                     # [B,1,36]

Host-side algebraic folds:
  1) x @ W1 = ub @ (Wb-Wc) + (q*ub) @ Wd + q @ (Wa+Wc); q is per-batch, so fold
     into per-batch weights Waug_b = [(Wb-Wc) + diag(q_b) Wd ; q_b(Wa+Wc)+b1]
     ([37,80]) and augment ub with a ones column -> single K=37 matmul.
  2) sigmoid(x) = 0.5 + 0.5*tanh(x/2); tanh and exp share one ACT table set,
     so the device computes t = tanh(pre/2) and the 0.5/0.5 affine is folded
     into the next layer's weights/biases.

Device strategy (v1, bf16):
  - Host pre-packs all DRAM arrays in the exact SBUF layouts so every DMA is
    a few large contiguous descriptors (the fp32 version spent ~775us of SP
    sequencer time generating gather descriptors and saturated DMA ring 0).
  - ubaug is shipped twice: transposed ([37, rows] per phase) feeding mm1
    directly (no on-chip transposes), and batch-aligned natural tiles
    ([128, 2, 37] per batch) feeding the final weighted sum.
  - All matmuls run in bf16 (psum fp32): mm1 streams 200 cols/batch, mm2
    400 cols per 4 batches, mm3 computes 4 batches in one 400-col stream via
    a [104,2] two-block W3, the weighted sum is data-stationary (ub tile as
    lhsT, softmax-weight columns as rhs) at 2 matmuls/batch.
  - Softmax weights are transposed on-chip (PE) instead of a DRAM roundtrip.
  - Work is software-pipelined so PE/ACT/DVE overlap; phase tail work (softmax,
    weighted sum of phase p) is interleaved into phase p+1's MLP groups.
"""

from contextlib import ExitStack

import numpy as np
import ml_dtypes

import concourse.bass as bass
import concourse.bacc as bacc
import concourse.tile as tile
from concourse import mybir
from concourse.bass_utils import run_bass_kernel_spmd

DEBUG_TAPS = False

B, T, E = 4096, 200, 36
N_CORES = 8
BC = B // N_CORES          # batches per core (512)
PB = 64                    # batches per phase
PH = BC // PB              # phases (8)
RP = PB * T                # rows per phase (12800)
F32 = mybir.dt.float32
BF16 = mybir.dt.bfloat16
AF = mybir.ActivationFunctionType
ALU = mybir.AluOpType
BF_NP = ml_dtypes.bfloat16


def dap(t, offset, dims):
    return bass.AP(tensor=t.tensor, offset=t.offset + offset, ap=dims)


def build_module():
    nc = bacc.Bacc(
        "TRN2", target_bir_lowering=False, debug=False,
        enable_asserts=False, num_devices=N_CORES,
    )

    # host-prepacked inputs (layouts match SBUF tiles; all DMAs are contiguous)
    ubt_d = nc.dram_tensor("ubt", [PH, 37, RP], BF16, kind="ExternalInput").ap()
    natb_d = nc.dram_tensor("natb", [PH, 128, PB * 2 * 37], BF16,
                            kind="ExternalInput").ap()
    waugt_d = nc.dram_tensor("waugt", [PH, 37, PB * 80], BF16,
                             kind="ExternalInput").ap()
    w2p_d = nc.dram_tensor("w2p", [80, 64], BF16, kind="ExternalInput").ap()
    w3p_d = nc.dram_tensor("w3p", [104, 2], BF16, kind="ExternalInput").ap()
    b2c_d = nc.dram_tensor("b2c", [128, 1], F32, kind="ExternalInput").ap()
    b3c_d = nc.dram_tensor("b3c", [64, 1], F32, kind="ExternalInput").ap()
    lens_d = nc.dram_tensor("lens", [PH, 64], F32, kind="ExternalInput").ap()
    out_d = nc.dram_tensor("out", [BC, 37], F32, kind="ExternalOutput").ap()
    if DEBUG_TAPS:
        sc_dram = nc.dram_tensor("sc_scratch", [BC * T], F32,
                                 kind="ExternalOutput").ap()
        w_dbg = nc.dram_tensor("w_dbg", [BC, T], F32, kind="ExternalOutput").ap()
        h1_dbg = nc.dram_tensor("h1_dbg", [80, 800], BF16,
                                kind="ExternalOutput").ap()
        h2_dbg = nc.dram_tensor("h2_dbg", [128, 400], BF16,
                                kind="ExternalOutput").ap()
        mv_dbg = nc.dram_tensor("mv_dbg", [37, 64], F32,
                                kind="ExternalOutput").ap()

    ident64_d = nc.inline_tensor(np.eye(64, dtype=np.float32), name="ident64").ap()
    identf_d = nc.inline_tensor(np.eye(37, dtype=np.float32), name="identf").ap()
    iota_d = nc.inline_tensor(
        np.broadcast_to(np.arange(T, dtype=np.float32), (64, T)).copy(),
        name="iotat").ap()

    with tile.TileContext(nc) as tc, ExitStack() as es:
        cpool = es.enter_context(tc.tile_pool(name="consts", bufs=1))
        ubtp = es.enter_context(tc.tile_pool(name="ubtp", bufs=2))
        natp = es.enter_context(tc.tile_pool(name="natp", bufs=3))
        waugp = es.enter_context(tc.tile_pool(name="waugp", bufs=2))
        h1p = es.enter_context(tc.tile_pool(name="h1p", bufs=3))
        h2p = es.enter_context(tc.tile_pool(name="h2p", bufs=3))
        scbp = es.enter_context(tc.tile_pool(name="scbp", bufs=2))
        smp = es.enter_context(tc.tile_pool(name="smp", bufs=2))
        wtp = es.enter_context(tc.tile_pool(name="wtp", bufs=2))
        mvsp = es.enter_context(tc.tile_pool(name="mvsp", bufs=2))
        otp = es.enter_context(tc.tile_pool(name="otp", bufs=2))
        m1p = es.enter_context(tc.tile_pool(name="m1p", bufs=2, space="PSUM"))
        m2p = es.enter_context(tc.tile_pool(name="m2p", bufs=2, space="PSUM"))
        scp = es.enter_context(tc.tile_pool(name="scp", bufs=1, space="PSUM"))
        smps = es.enter_context(tc.tile_pool(name="smps", bufs=1, space="PSUM"))

        ident64 = cpool.tile([64, 64], F32)
        nc.sync.dma_start(out=ident64, in_=ident64_d)
        identf = cpool.tile([37, 37], F32)
        nc.sync.dma_start(out=identf, in_=identf_d)
        iota_t = cpool.tile([64, T], F32)
        nc.sync.dma_start(out=iota_t, in_=iota_d)
        w2_t = cpool.tile([80, 64], BF16)
        nc.sync.dma_start(out=w2_t, in_=w2p_d)
        w3_t = cpool.tile([104, 2], BF16)
        nc.sync.dma_start(out=w3_t, in_=w3p_d)
        b2_t = cpool.tile([128, 1], F32)
        nc.sync.dma_start(out=b2_t, in_=b2c_d)
        b3_t = cpool.tile([64, 1], F32)
        nc.sync.dma_start(out=b3_t, in_=b3c_d)
        lensall_t = cpool.tile([64, PH], F32)
        nc.sync.dma_start(
            out=lensall_t,
            in_=dap(lens_d, 0, [[1, 64], [64, PH]]))

        loaded = {}

        def emit_loads(ph):
            ubt_t = ubtp.tile([37, RP], BF16, tag="ubt", name=f"ubt{ph}")
            nc.scalar.dma_start(
                out=ubt_t, in_=dap(ubt_d, ph * 37 * RP, [[RP, 37], [1, RP]]))
            nat_t = natp.tile([128, PB, 2, 37], BF16, tag="nat", name=f"nat{ph}")
            nc.scalar.dma_start(
                out=nat_t,
                in_=dap(natb_d, ph * 128 * PB * 2 * 37,
                        [[PB * 2 * 37, 128], [1, PB * 2 * 37]]))
            waug_t = waugp.tile([37, PB, 80], BF16, tag="waug", name=f"waug{ph}")
            nc.scalar.dma_start(
                out=waug_t,
                in_=dap(waugt_d, ph * 37 * PB * 80,
                        [[PB * 80, 37], [1, PB * 80]]))
            loaded[ph] = (ubt_t, nat_t, waug_t, lensall_t[:, ph:ph + 1])

        def emit_wt_transposes(ph):
            """Transpose softmax weights of phase ph for the weighted sum."""
            wb = loaded[ph + 100]  # wb tile stored under key ph+100
            smt = smps.tile([128, 128], F32, tag="sm", name=f"smt{ph}")
            nc.tensor.transpose(smt[0:128, 0:64], wb[:, 0:128], ident64)
            nc.tensor.transpose(smt[0:72, 64:128], wb[:, 128:200], ident64)
            wT0 = wtp.tile([128, 64], BF16, tag="wt0", name=f"wt0{ph}")
            nc.vector.tensor_copy(out=wT0, in_=smt[0:128, 0:64])
            wT1 = wtp.tile([72, 64], BF16, tag="wt1", name=f"wt1{ph}")
            nc.vector.tensor_copy(out=wT1, in_=smt[0:72, 64:128])
            loaded[ph + 200] = (smt, wT0, wT1)

        def emit_mv(ph, b0, b1):
            """Weighted-sum matmuls for batches [b0, b1) of phase ph."""
            nat_t = loaded[ph][1]
            smt, wT0, wT1 = loaded[ph + 200]
            for b in range(b0, b1):
                nc.tensor.matmul(
                    smt[0:37, b:b + 1], nat_t[:, b, 0, :], wT0[:, b:b + 1],
                    start=True, stop=False)
                nc.tensor.matmul(
                    smt[0:37, b:b + 1], nat_t[0:72, b, 1, :], wT1[:, b:b + 1],
                    start=False, stop=True)

        def emit_out(ph):
            smt, _, _ = loaded[ph + 200]
            mvs = mvsp.tile([37, 64], F32, tag="mvs", name=f"mvs{ph}")
            nc.vector.tensor_copy(out=mvs, in_=smt[0:37, 0:64])
            if DEBUG_TAPS and ph == 0:
                nc.sync.dma_start(out=mv_dbg, in_=mvs)
            nc.tensor.transpose(smt[0:64, 64:101], mvs, identf)
            ot = otp.tile([64, 37], F32, tag="ot", name=f"ot{ph}")
            nc.vector.tensor_copy(out=ot, in_=smt[0:64, 64:101])
            nc.sync.dma_start(
                out=dap(out_d, 37 * PB * ph, [[37, 64], [1, 37]]),
                in_=ot)

        def emit_softmax(ph, sc_t):
            lens_t = loaded[ph][3]
            if DEBUG_TAPS:
                nc.sync.dma_start(
                    out=dap(sc_dram, T * PB * ph, [[T, 64], [1, T]]), in_=sc_t)
            mask_t = smp.tile([64, T], F32, tag="mask", name=f"mask{ph}")
            nc.vector.tensor_scalar(
                out=mask_t, in0=iota_t, scalar1=lens_t, scalar2=None,
                op0=ALU.is_lt)
            masked = smp.tile([64, T], F32, tag="masked", name=f"masked{ph}")
            nc.vector.scalar_tensor_tensor(
                out=masked, in0=sc_t, scalar=b3_t, in1=mask_t,
                op0=ALU.add, op1=ALU.mult)
            negmax = smp.tile([64, 1], F32, tag="negmax", name=f"negmax{ph}")
            nc.vector.tensor_reduce(
                out=negmax, in_=masked, axis=mybir.AxisListType.X,
                op=ALU.max, negate=True)
            ew = smp.tile([64, T], F32, tag="ew", name=f"ew{ph}")
            sumexp = smp.tile([64, 1], F32, tag="sumexp", name=f"sumexp{ph}")
            nc.scalar.activation(
                out=ew, in_=masked, func=AF.Exp, bias=negmax, accum_out=sumexp)
            rz = smp.tile([64, 1], F32, tag="rz", name=f"rz{ph}")
            nc.vector.reciprocal(rz, sumexp)
            w_t = smp.tile([64, T], F32, tag="wt", name=f"wt{ph}")
            nc.vector.tensor_scalar_mul(w_t, ew, rz)
            if DEBUG_TAPS:
                nc.sync.dma_start(
                    out=dap(w_dbg, T * PB * ph, [[T, 64], [1, T]]), in_=w_t)
            loaded[ph + 100] = w_t

        emit_loads(0)
        for ph in range(PH):
            if ph + 1 < PH:
                emit_loads(ph + 1)
            ubt_t, nat_t, waug_t, lens_t = loaded[ph]
            prev = ph - 1 if ph > 0 else None

            m1_tiles = {}
            h1_tiles = {}
            h2_tiles = {}
            sc_tile = [None]
            sc_t_phase = [smp.tile([64, T], F32, tag="sct", name=f"sct{ph}")]

            # 16 groups of 4 batches + 2 drain iterations, software-pipelined:
            # PE order per iter: mm1(g), [tail work of prev phase], mm2(g-1),
            # mm3(g-2).  ACT order: h1(g), h2(g-1).
            for g in range(18):
                if g < 16:
                    m1_ps = m1p.tile([80, 1024], F32, tag="m1", name=f"m1_{ph}_{g}")
                    for j in range(4):
                        b = 4 * g + j
                        colbase = 512 * (j // 2) + 200 * (j % 2)
                        nc.tensor.matmul(
                            m1_ps[0:80, colbase:colbase + 200],
                            waug_t[:, b, :],
                            ubt_t[:, 200 * b:200 * b + 200],
                            start=True, stop=True)
                    h1_t = h1p.tile([80, 2, 400], BF16, tag="h1", name=f"h1_{ph}_{g}")
                    nc.scalar.activation(
                        out=h1_t,
                        in_=m1_ps.rearrange("p (u c) -> p u c", u=2)[:, :, 0:400],
                        func=AF.Tanh, scale=0.5)
                    m1_tiles[g] = m1_ps
                    h1_tiles[g] = h1_t
                    if DEBUG_TAPS and ph == 0 and g == 0:
                        nc.sync.dma_start(
                            out=dap(h1_dbg, 0, [[800, 80], [1, 800]]),
                            in_=dap(h1_t, 0, [[800, 80], [1, 800]]))

                if prev is not None:
                    if g == 2:
                        emit_wt_transposes(prev)
                    if 3 <= g <= 15:
                        b0 = 5 * (g - 3)
                        b1 = min(64, 5 * (g - 2))
                        emit_mv(prev, b0, b1)
                    if g == 16:
                        emit_out(prev)

                if 1 <= g <= 16:
                    g1 = g - 1
                    h1_t = h1_tiles.pop(g1)
                    m2_ps = m2p.tile([128, 400], F32, tag="m2", name=f"m2_{ph}_{g1}")
                    for u in range(2):
                        nc.tensor.matmul(
                            m2_ps[64 * u:64 * u + 64, 0:400], w2_t,
                            h1_t[:, u, :], start=True, stop=True)
                    h2_t = h2p.tile([128, 400], BF16, tag="h2", name=f"h2_{ph}_{g1}")
                    nc.scalar.activation(
                        out=h2_t, in_=m2_ps, func=AF.Tanh, bias=b2_t, scale=0.5)
                    h2_tiles[g1] = h2_t
                    m1_tiles.pop(g1, None)
                    if DEBUG_TAPS and ph == 0 and g1 == 0:
                        nc.sync.dma_start(
                            out=dap(h2_dbg, 0, [[400, 128], [1, 400]]),
                            in_=dap(h2_t, 0, [[400, 128], [1, 400]]))

                if 2 <= g <= 17:
                    g2 = g - 2
                    q = g2 % 4
                    if q == 0:
                        sc_tile[0] = scp.tile([98, 400], F32, tag="sc",
                                              name=f"sc_{ph}_{g2 // 4}")
                    h2_t = h2_tiles.pop(g2)
                    # batch 4q+2*blk+hc -> psum partition 32q+blk, col half hc
                    nc.tensor.matmul(
                        sc_tile[0][32 * q:32 * q + 2, 0:400], w3_t,
                        h2_t[0:104, 0:400], start=True, stop=True,
                        tile_position=(0, 32 * q))
                    if q == 3:
                        g16 = g2 // 4
                        scb = scbp.tile([98, 400], F32, tag="scb",
                                        name=f"scb_{ph}_{g16}")
                        nc.vector.tensor_copy(out=scb, in_=sc_tile[0])
                        # SBUF->SBUF compaction into batch-major sc_t rows.
                        # SBUF AP rule: only the first dim may cross
                        # partitions (stride = k*pitch); blk/hc go into the
                        # offsets (offset = partition*pitch + col)
                        for blk in range(2):
                            for hc in range(2):
                                nc.sync.dma_start(
                                    out=dap(sc_t_phase[0],
                                            T * (16 * g16 + 2 * blk + hc),
                                            [[800, 4], [1, 200]]),
                                    in_=dap(scb, 400 * blk + 200 * hc,
                                            [[400 * 32, 4], [1, 200]]))

            emit_softmax(ph, sc_t_phase[0])

        # tail: softmax-weighted sum for the last phase
        emit_wt_transposes(PH - 1)
        emit_mv(PH - 1, 0, 64)
        emit_out(PH - 1)

    nc.compile()
    return nc


def host_prep(query_ad, user_behavior, user_behavior_length,
              W1, b1, W2, b2, W3, b3):
    q = np.asarray(query_ad, dtype=np.float32)
    ub = np.asarray(user_behavior, dtype=np.float32)
    lens = np.asarray(user_behavior_length)
    W1 = np.asarray(W1, dtype=np.float32)
    b1 = np.asarray(b1, dtype=np.float32)
    W2 = np.asarray(W2, dtype=np.float32)
    b2 = np.asarray(b2, dtype=np.float32)
    W3 = np.asarray(W3, dtype=np.float32)
    b3 = np.asarray(b3, dtype=np.float32)
    nb = q.shape[0]

    Wa, Wb, Wc, Wd = W1[0:36], W1[36:72], W1[72:108], W1[108:144]
    waug = np.empty((nb, 37, 80), dtype=np.float32)
    waug[:, 0:36, :] = (Wb - Wc)[None, :, :] + q[:, :, None] * Wd[None, :, :]
    waug[:, 36, :] = q @ (Wa + Wc) + b1[None, :]

    ubaug = np.empty((nb, T, 37), dtype=np.float32)
    ubaug[:, :, 0:36] = ub
    ubaug[:, :, 36] = 1.0

    # sigmoid -> tanh fold: h = 0.5 + 0.5*t with t = tanh(pre/2)
    w2f = 0.5 * W2
    b2f = 0.5 * (b2 + 0.5 * W2.sum(axis=0))
    w3f = 0.5 * W3
    b3f = float(b3[0] + 0.5 * W3.sum())

    w2p = np.zeros((80, 64), dtype=np.float32)
    w2p[:, 0:40] = w2f
    w3p = np.zeros((104, 2), dtype=np.float32)
    w3p[0:40, 0] = w3f[:, 0]
    w3p[64:104, 1] = w3f[:, 0]
    b2c = np.zeros((128, 1), dtype=np.float32)
    b2c[0:40, 0] = b2f
    b2c[64:104, 0] = b2f
    b3c = np.full((64, 1), b3f, dtype=np.float32)

    w2p = w2p.astype(BF_NP)
    w3p = w3p.astype(BF_NP)

    n_cores = nb // BC
    in_maps = []
    for c in range(n_cores):
        sl = slice(BC * c, BC * (c + 1))
        ub_c = ubaug[sl]                                    # [512, 200, 37]
        # mm1 rhs: transposed rows per phase [PH, 37, RP]
        ubt = np.ascontiguousarray(
            ub_c.reshape(PH, RP, 37).transpose(0, 2, 1)).astype(BF_NP)
        # weighted-sum lhsT: batch-aligned natural tiles [PH, 128, PB, 2, 37]
        pad = np.zeros((BC, 256, 37), dtype=np.float32)
        pad[:, 0:T] = ub_c
        natb = np.ascontiguousarray(
            pad.reshape(PH, PB, 2, 128, 37).transpose(0, 3, 1, 2, 4)
        ).reshape(PH, 128, PB * 2 * 37).astype(BF_NP)
        waugt = np.ascontiguousarray(
            waug[sl].reshape(PH, PB, 37, 80).transpose(0, 2, 1, 3)
        ).reshape(PH, 37, PB * 80).astype(BF_NP)
        lensc = lens[sl].astype(np.float32).reshape(PH, PB)
        in_maps.append({
            "ubt": ubt, "natb": natb, "waugt": waugt,
            "w2p": w2p, "w3p": w3p, "b2c": b2c, "b3c": b3c,
            "lens": lensc,
        })
    return in_maps


_NC_CACHE = {}


def get_module():
    if "nc" not in _NC_CACHE:
        _NC_CACHE["nc"] = build_module()
    return _NC_CACHE["nc"]


def kernel(query_ad, user_behavior, user_behavior_length,
           W1, b1, W2, b2, W3, b3, trace=False):
    nc = get_module()
    in_maps = host_prep(query_ad, user_behavior, user_behavior_length,
                        W1, b1, W2, b2, W3, b3)
    res = run_bass_kernel_spmd(nc, in_maps, core_ids=list(range(N_CORES)),
                               trace=trace)
    outs = [np.asarray(res.results[c]["out"])[:, 0:36] for c in range(N_CORES)]
    full = np.ascontiguousarray(np.concatenate(outs, axis=0)).reshape(B, 1, 36)
    kernel.last_results = res.results
    if trace:
        kernel.last_result = res
    return full


# revision 69
# speedup vs baseline: 2.9623x; 1.3548x over previous
"""DIN attention layer kernel for Trainium2 (8 NeuronCores, data-parallel over batch).

Reference math:
  x  = concat([q, ub, q-ub, q*ub], -1)             # [B,T,144]
  h1 = sigmoid(x @ W1 + b1)                        # [B,T,80]
  h2 = sigmoid(h1 @ W2 + b2)                       # [B,T,40]
  s  = h2 @ W3 + b3                                # [B,T,1]
  w  = softmax(s.T * mask)                         # [B,1,T]  (multiplicative mask)
  out = w @ ub                                     # [B,1,36]

Host-side algebraic folds:
  1) x @ W1 = ub @ (Wb-Wc) + (q*ub) @ Wd + q @ (Wa+Wc); q is per-batch, so fold
     into per-batch weights Waug_b = [(Wb-Wc) + diag(q_b) Wd ; q_b(Wa+Wc)+b1]
     ([37,80]) and augment ub with a ones column -> single K=37 matmul.
  2) sigmoid(x) = 0.5 + 0.5*tanh(x/2); tanh and exp share one ACT table set,
     so the device computes t = tanh(pre/2) and the 0.5/0.5 affine is folded
     into the next layer's weights/biases.

Device strategy (bf16, 388us vs 1156us fp32 baseline):
  - Host pre-packs every DRAM array in the exact SBUF layout so each load is
    one contiguous DMA (the fp32 baseline spent ~775us of SP sequencer time
    generating gather descriptors).  All big loads use 128-partition tiles:
    descriptors of <=64-partition loads all execute on ONE DMA engine
    (22.5 GB/s) while 128-partition loads spread across all 16 engines —
    this alone was worth ~140us.  ubt/waug stack two 32-batch blocks at
    partition bases 0/64 (matmul operands may sit at base 64).
  - ubaug ships twice: transposed (feeding mm1 directly, no on-chip
    transposes) and batch-aligned natural tiles (feeding the weighted sum).
  - All matmuls bf16 (psum fp32): mm1 200 cols/batch, mm2 400 cols per 4
    batches, mm3 computes 4 batches per 400-col stream via a [104,2]
    two-block W3, then a [98,8] selection matmul compacts the scattered
    psum partitions into dense rows (batch = 16*(p//32) + 2*(p%32) + hc) —
    partition moves are only possible on PE or DMA, and per-descriptor DMA
    was the previous bottleneck.  Softmax runs on that psum layout
    directly; softmax weights are PE-transposed for the data-stationary
    weighted sum (ub tile as lhsT, weight columns as rhs, 2 matmuls/batch).
  - Work is software-pipelined: phase p's softmax + weighted sum interleave
    into phase p+1's MLP groups so PE/ACT/DVE overlap (PE ~80% busy).
  Pitfalls baked into the structure: psum matmul accumulation chains must
  not interleave with other matmuls (corruption); SBUF APs may cross
  partitions only in the first dim (offset = partition*pitch + col); the
  AP balancer must not be allowed to merge a partition-crossing dim with
  an inner dim (keep them non-mergeable or split calls).
"""

from contextlib import ExitStack

import numpy as np
import ml_dtypes

import concourse.bass as bass
import concourse.bacc as bacc
import concourse.tile as tile
from concourse import mybir
from concourse.bass_utils import run_bass_kernel_spmd

DEBUG_TAPS = False

B, T, E = 4096, 200, 36
N_CORES = 8
BC = B // N_CORES          # batches per core (512)
PB = 64                    # batches per phase
PH = BC // PB              # phases (8)
RP = PB * T                # rows per phase (12800)
F32 = mybir.dt.float32
BF16 = mybir.dt.bfloat16
AF = mybir.ActivationFunctionType
ALU = mybir.AluOpType
BF_NP = ml_dtypes.bfloat16


def dap(t, offset, dims):
    return bass.AP(tensor=t.tensor, offset=t.offset + offset, ap=dims)


def build_module():
    nc = bacc.Bacc(
        "TRN2", target_bir_lowering=False, debug=False,
        enable_asserts=False, num_devices=N_CORES,
    )

    # host-prepacked inputs (layouts match SBUF tiles; all DMAs are contiguous)
    ubt_d = nc.dram_tensor("ubt", [PH, 128, RP // 2], BF16,
                           kind="ExternalInput").ap()
    natb_d = nc.dram_tensor("natb", [PH, 128, PB * 2 * 37], BF16,
                            kind="ExternalInput").ap()
    waugt_d = nc.dram_tensor("waugt", [PH, 128, PB * 40], BF16,
                             kind="ExternalInput").ap()
    w2p_d = nc.dram_tensor("w2p", [80, 64], BF16, kind="ExternalInput").ap()
    w3p_d = nc.dram_tensor("w3p", [104, 2], BF16, kind="ExternalInput").ap()
    b2c_d = nc.dram_tensor("b2c", [128, 1], F32, kind="ExternalInput").ap()
    b3c_d = nc.dram_tensor("b3c", [104, 1], F32, kind="ExternalInput").ap()
    lens_d = nc.dram_tensor("lens", [104, 2 * PH], F32,
                            kind="ExternalInput").ap()
    out_d = nc.dram_tensor("out", [BC, 37], F32, kind="ExternalOutput").ap()
    if DEBUG_TAPS:
        sc_dram = nc.dram_tensor("sc_scratch", [BC * T], F32,
                                 kind="ExternalOutput").ap()
        w_dbg = nc.dram_tensor("w_dbg", [PH, 104, 400], F32,
                               kind="ExternalOutput").ap()
        h1_dbg = nc.dram_tensor("h1_dbg", [80, 800], BF16,
                                kind="ExternalOutput").ap()
        h2_dbg = nc.dram_tensor("h2_dbg", [128, 400], BF16,
                                kind="ExternalOutput").ap()
        mv_dbg = nc.dram_tensor("mv_dbg", [37, 64], F32,
                                kind="ExternalOutput").ap()

    ident104_d = nc.inline_tensor(np.eye(104, dtype=np.float32),
                                  name="ident104").ap()
    identf_d = nc.inline_tensor(np.eye(37, dtype=np.float32), name="identf").ap()
    iota_d = nc.inline_tensor(
        np.broadcast_to(np.tile(np.arange(T, dtype=np.float32), 2),
                        (104, 2 * T)).copy(),
        name="iotat").ap()
    sel = np.zeros((98, 8), dtype=np.float32)
    for qq in range(4):
        for blk in range(2):
            sel[32 * qq + blk, 2 * qq + blk] = 1.0
    sel_d = nc.inline_tensor(sel.astype(BF_NP), name="sel98").ap()

    with tile.TileContext(nc) as tc, ExitStack() as es:
        cpool = es.enter_context(tc.tile_pool(name="consts", bufs=1))
        ubtp = es.enter_context(tc.tile_pool(name="ubtp", bufs=2))
        natp = es.enter_context(tc.tile_pool(name="natp", bufs=3))
        waugp = es.enter_context(tc.tile_pool(name="waugp", bufs=2))
        h1p = es.enter_context(tc.tile_pool(name="h1p", bufs=3))
        h2p = es.enter_context(tc.tile_pool(name="h2p", bufs=3))
        scbp = es.enter_context(tc.tile_pool(name="scbp", bufs=2))
        smp = es.enter_context(tc.tile_pool(name="smp", bufs=2))
        wtp = es.enter_context(tc.tile_pool(name="wtp", bufs=2))
        mvsp = es.enter_context(tc.tile_pool(name="mvsp", bufs=2))
        otp = es.enter_context(tc.tile_pool(name="otp", bufs=2))
        m1p = es.enter_context(tc.tile_pool(name="m1p", bufs=3, space="PSUM"))
        m2p = es.enter_context(tc.tile_pool(name="m2p", bufs=2, space="PSUM"))
        scp = es.enter_context(tc.tile_pool(name="scp", bufs=1, space="PSUM"))
        sc2p = es.enter_context(tc.tile_pool(name="sc2p", bufs=1, space="PSUM"))
        smps = es.enter_context(tc.tile_pool(name="smps", bufs=1, space="PSUM"))

        ident104 = cpool.tile([104, 104], F32)
        nc.sync.dma_start(out=ident104, in_=ident104_d)
        identf = cpool.tile([37, 37], F32)
        nc.sync.dma_start(out=identf, in_=identf_d)
        iota_t = cpool.tile([104, 2 * T], F32)
        nc.sync.dma_start(out=iota_t, in_=iota_d)
        sel_t = cpool.tile([98, 8], BF16)
        nc.sync.dma_start(out=sel_t, in_=sel_d)
        w2_t = cpool.tile([80, 64], BF16)
        nc.sync.dma_start(out=w2_t, in_=w2p_d)
        w3_t = cpool.tile([104, 2], BF16)
        nc.sync.dma_start(out=w3_t, in_=w3p_d)
        b2_t = cpool.tile([128, 1], F32)
        nc.sync.dma_start(out=b2_t, in_=b2c_d)
        b3_t = cpool.tile([104, 1], F32)
        nc.sync.dma_start(out=b3_t, in_=b3c_d)
        lensall_t = cpool.tile([104, 2 * PH], F32)
        nc.sync.dma_start(out=lensall_t, in_=lens_d)

        loaded = {}

        def emit_loads(ph):
            # ubt/waug have only 37 partitions, so their per-partition-line
            # descriptors are huge and serialize on one DMA engine; split
            # into column chunks alternated across the two HWDGE queues
            # ubt/waug ship as 128-partition tiles (two 32-batch blocks at
            # partition bases 0 and 64, zero-padded rows between): DMA
            # descriptors for 128-partition contiguous loads spread across
            # all 16 DMA engines, while <=64-partition loads serialize on one
            ubt_t = ubtp.tile([128, RP // 2], BF16, tag="ubt", name=f"ubt{ph}")
            nc.scalar.dma_start(
                out=ubt_t, in_=dap(ubt_d, ph * 128 * (RP // 2),
                                   [[RP // 2, 128], [1, RP // 2]]))
            nat_t = natp.tile([128, PB, 2, 37], BF16, tag="nat", name=f"nat{ph}")
            nc.scalar.dma_start(
                out=nat_t,
                in_=dap(natb_d, ph * 128 * PB * 2 * 37,
                        [[PB * 2 * 37, 128], [1, PB * 2 * 37]]))
            waug_t = waugp.tile([128, PB // 2, 80], BF16, tag="waug",
                                name=f"waug{ph}")
            nc.sync.dma_start(
                out=waug_t,
                in_=dap(waugt_d, ph * 128 * PB * 40,
                        [[PB * 40, 128], [1, PB * 40]]))
            loaded[ph] = (ubt_t, nat_t, waug_t, lensall_t[:, ph:ph + 1])

        def emit_wt_transposes(ph):
            """Transpose softmax weights of phase ph for the weighted sum."""
            wb = loaded[ph + 100]  # w_t [104, 2, 200]
            smt = smps.tile([128, 128], F32, tag="sm", name=f"smt{ph}")
            wT = {}
            for hc in range(2):
                nc.tensor.transpose(
                    smt[0:128, 0:104], wb[:, hc, 0:128], ident104)
                wT0 = wtp.tile([128, 104], BF16, tag=f"wt0{hc}",
                               name=f"wt0{hc}_{ph}")
                nc.vector.tensor_copy(out=wT0, in_=smt[0:128, 0:104])
                nc.tensor.transpose(
                    smt[0:72, 0:104], wb[:, hc, 128:200], ident104)
                wT1 = wtp.tile([72, 104], BF16, tag=f"wt1{hc}",
                               name=f"wt1{hc}_{ph}")
                nc.vector.tensor_copy(out=wT1, in_=smt[0:72, 0:104])
                wT[hc] = (wT0, wT1)
            loaded[ph + 200] = (smt, wT)

        def emit_mv(ph, b0, b1):
            """Weighted-sum matmuls for batches [b0, b1) of phase ph."""
            nat_t = loaded[ph][1]
            smt, wT = loaded[ph + 200]
            for b in range(b0, b1):
                hc = b % 2
                col = 32 * (b // 16) + (b % 16) // 2
                wT0, wT1 = wT[hc]
                nc.tensor.matmul(
                    smt[0:37, b:b + 1], nat_t[:, b, 0, :],
                    wT0[:, col:col + 1], start=True, stop=False)
                nc.tensor.matmul(
                    smt[0:37, b:b + 1], nat_t[0:72, b, 1, :],
                    wT1[:, col:col + 1], start=False, stop=True)

        def emit_out(ph):
            smt, _ = loaded[ph + 200]
            mvs = mvsp.tile([37, 64], F32, tag="mvs", name=f"mvs{ph}")
            nc.vector.tensor_copy(out=mvs, in_=smt[0:37, 0:64])
            if DEBUG_TAPS and ph == 0:
                nc.sync.dma_start(out=mv_dbg, in_=mvs)
            nc.tensor.transpose(smt[0:64, 64:101], mvs, identf)
            ot = otp.tile([64, 37], F32, tag="ot", name=f"ot{ph}")
            nc.vector.tensor_copy(out=ot, in_=smt[0:64, 64:101])
            nc.sync.dma_start(
                out=dap(out_d, 37 * PB * ph, [[37, 64], [1, 37]]),
                in_=ot)

        def emit_softmax(ph, sc_t):
            # sc_t: [104, 2, 200] psum; batch = 16*(p//32) + 2*(p%32) + hc
            mask_t = smp.tile([104, 2, T], F32, tag="mask", name=f"mask{ph}")
            for hc in range(2):
                nc.vector.tensor_scalar(
                    out=mask_t[:, hc, :], in0=iota_t[:, 0:200],
                    scalar1=lensall_t[:, 2 * ph + hc:2 * ph + hc + 1],
                    scalar2=None, op0=ALU.is_lt)
            masked = smp.tile([104, 2, T], F32, tag="masked", name=f"masked{ph}")
            nc.vector.scalar_tensor_tensor(
                out=masked, in0=sc_t.rearrange("p (u c) -> p u c", u=2),
                scalar=b3_t, in1=mask_t, op0=ALU.add, op1=ALU.mult)
            negmax = smp.tile([104, 2], F32, tag="negmax", name=f"negmax{ph}")
            nc.vector.tensor_reduce(
                out=negmax, in_=masked, axis=mybir.AxisListType.X,
                op=ALU.max, negate=True)
            ew = smp.tile([104, 2, T], F32, tag="ew", name=f"ew{ph}")
            sumexp = smp.tile([104, 2], F32, tag="sumexp", name=f"sumexp{ph}")
            for hc in range(2):
                nc.scalar.activation(
                    out=ew[:, hc, :], in_=masked[:, hc, :], func=AF.Exp,
                    bias=negmax[:, hc:hc + 1],
                    accum_out=sumexp[:, hc:hc + 1])
            rz = smp.tile([104, 2], F32, tag="rz", name=f"rz{ph}")
            nc.vector.reciprocal(rz, sumexp)
            w_t = smp.tile([104, 2, T], F32, tag="wt", name=f"wt{ph}")
            for hc in range(2):
                nc.vector.tensor_scalar_mul(
                    w_t[:, hc, :], ew[:, hc, :], rz[:, hc:hc + 1])
            if DEBUG_TAPS:
                nc.sync.dma_start(
                    out=dap(w_dbg, 2 * T * 104 * ph, [[400, 104], [1, 400]]),
                    in_=w_t)
            loaded[ph + 100] = w_t

        emit_loads(0)
        for ph in range(PH):
            if ph + 1 < PH:
                emit_loads(ph + 1)
            ubt_t, nat_t, waug_t, lens_t = loaded[ph]
            prev = ph - 1 if ph > 0 else None

            m1_tiles = {}
            h1_tiles = {}
            h2_tiles = {}
            sc_tile = [None]
            sc_t_phase = [sc2p.tile([104, 400], F32, tag="sc2",
                                    name=f"sc2_{ph}")]

            # 16 groups of 4 batches + 2 drain iterations, software-pipelined:
            # PE order per iter: mm1(g), [tail work of prev phase], mm2(g-1),
            # mm3(g-2).  ACT order: h1(g), h2(g-1).
            for g in range(18):
                if g < 16:
                    h1_t = h1p.tile([80, 2, 400], BF16, tag="h1", name=f"h1_{ph}_{g}")
                    for u in range(2):
                        m1_ps = m1p.tile([80, 512], F32, tag="m1",
                                         name=f"m1_{ph}_{g}_{u}")
                        for j in range(2):
                            b = 4 * g + 2 * u + j
                            base = 64 * (b // 32)
                            bl = b % 32
                            nc.tensor.matmul(
                                m1_ps[0:80, 200 * j:200 * j + 200],
                                waug_t[base:base + 37, bl, :],
                                ubt_t[base:base + 37,
                                      200 * bl:200 * bl + 200],
                                start=True, stop=True)
                        nc.scalar.activation(
                            out=h1_t[:, u, :], in_=m1_ps[0:80, 0:400],
                            func=AF.Tanh, scale=0.5)
                    h1_tiles[g] = h1_t
                    if DEBUG_TAPS and ph == 0 and g == 0:
                        nc.sync.dma_start(
                            out=dap(h1_dbg, 0, [[800, 80], [1, 800]]),
                            in_=dap(h1_t, 0, [[800, 80], [1, 800]]))

                if prev is not None:
                    if g == 6:
                        emit_wt_transposes(prev)
                    if 7 <= g <= 16:
                        b0 = 7 * (g - 7)
                        b1 = min(64, 7 * (g - 6))
                        emit_mv(prev, b0, b1)
                    if g == 17:
                        emit_out(prev)

                if 1 <= g <= 16:
                    g1 = g - 1
                    h1_t = h1_tiles.pop(g1)
                    m2_ps = m2p.tile([128, 400], F32, tag="m2", name=f"m2_{ph}_{g1}")
                    for u in range(2):
                        nc.tensor.matmul(
                            m2_ps[64 * u:64 * u + 64, 0:400], w2_t,
                            h1_t[:, u, :], start=True, stop=True)
                    h2_t = h2p.tile([128, 400], BF16, tag="h2", name=f"h2_{ph}_{g1}")
                    nc.scalar.activation(
                        out=h2_t, in_=m2_ps, func=AF.Tanh, bias=b2_t, scale=0.5)
                    h2_tiles[g1] = h2_t
                    m1_tiles.pop(g1, None)
                    if DEBUG_TAPS and ph == 0 and g1 == 0:
                        nc.sync.dma_start(
                            out=dap(h2_dbg, 0, [[400, 128], [1, 400]]),
                            in_=dap(h2_t, 0, [[400, 128], [1, 400]]))

                if 2 <= g <= 17:
                    g2 = g - 2
                    q = g2 % 4
                    if q == 0:
                        sc_tile[0] = scp.tile([98, 400], F32, tag="sc",
                                              name=f"sc_{ph}_{g2 // 4}")
                    h2_t = h2_tiles.pop(g2)
                    # batch 4q+2*blk+hc -> psum partition 32q+blk, col half hc
                    nc.tensor.matmul(
                        sc_tile[0][32 * q:32 * q + 2, 0:400], w3_t,
                        h2_t[0:104, 0:400], start=True, stop=True,
                        tile_position=(0, 32 * q))
                    if q == 3:
                        g16 = g2 // 4
                        scb = scbp.tile([98, 400], BF16, tag="scb",
                                        name=f"scb_{ph}_{g16}")
                        nc.vector.tensor_copy(out=scb, in_=sc_tile[0])
                        # PE compaction: sel maps partition 32q+blk to dense
                        # row 2q+blk; per-g16 rows land at psum 32*g16+0:8,
                        # so batch = 16*(p//32) + 2*(p%32) + hc in sc2
                        nc.tensor.matmul(
                            sc_t_phase[0][32 * g16:32 * g16 + 8, 0:400],
                            sel_t, scb, start=True, stop=True,
                            tile_position=(0, 32 * g16))

            emit_softmax(ph, sc_t_phase[0])

        # tail: softmax-weighted sum for the last phase
        emit_wt_transposes(PH - 1)
        emit_mv(PH - 1, 0, 64)
        emit_out(PH - 1)

    nc.compile()
    return nc


def host_prep(query_ad, user_behavior, user_behavior_length,
              W1, b1, W2, b2, W3, b3):
    q = np.asarray(query_ad, dtype=np.float32)
    ub = np.asarray(user_behavior, dtype=np.float32)
    lens = np.asarray(user_behavior_length)
    W1 = np.asarray(W1, dtype=np.float32)
    b1 = np.asarray(b1, dtype=np.float32)
    W2 = np.asarray(W2, dtype=np.float32)
    b2 = np.asarray(b2, dtype=np.float32)
    W3 = np.asarray(W3, dtype=np.float32)
    b3 = np.asarray(b3, dtype=np.float32)
    nb = q.shape[0]

    Wa, Wb, Wc, Wd = W1[0:36], W1[36:72], W1[72:108], W1[108:144]
    waug = np.empty((nb, 37, 80), dtype=np.float32)
    waug[:, 0:36, :] = (Wb - Wc)[None, :, :] + q[:, :, None] * Wd[None, :, :]
    waug[:, 36, :] = q @ (Wa + Wc) + b1[None, :]

    ubaug = np.empty((nb, T, 37), dtype=np.float32)
    ubaug[:, :, 0:36] = ub
    ubaug[:, :, 36] = 1.0

    # sigmoid -> tanh fold: h = 0.5 + 0.5*t with t = tanh(pre/2)
    w2f = 0.5 * W2
    b2f = 0.5 * (b2 + 0.5 * W2.sum(axis=0))
    w3f = 0.5 * W3
    b3f = float(b3[0] + 0.5 * W3.sum())

    w2p = np.zeros((80, 64), dtype=np.float32)
    w2p[:, 0:40] = w2f
    w3p = np.zeros((104, 2), dtype=np.float32)
    w3p[0:40, 0] = w3f[:, 0]
    w3p[64:104, 1] = w3f[:, 0]
    b2c = np.zeros((128, 1), dtype=np.float32)
    b2c[0:40, 0] = b2f
    b2c[64:104, 0] = b2f
    b3c = np.full((104, 1), b3f, dtype=np.float32)

    w2p = w2p.astype(BF_NP)
    w3p = w3p.astype(BF_NP)

    n_cores = nb // BC
    in_maps = []
    for c in range(n_cores):
        sl = slice(BC * c, BC * (c + 1))
        ub_c = ubaug[sl]                                    # [512, 200, 37]
        # mm1 rhs: transposed rows, two 32-batch blocks stacked at
        # partition bases 0/64 -> [PH, 128, RP/2]
        ubt_r = ub_c.reshape(PH, 2, RP // 2, 37).transpose(0, 3, 1, 2)
        ubt = np.zeros((PH, 128, RP // 2), dtype=np.float32)
        ubt[:, 0:37] = ubt_r[:, :, 0]
        ubt[:, 64:101] = ubt_r[:, :, 1]
        ubt = ubt.astype(BF_NP)
        # weighted-sum lhsT: batch-aligned natural tiles [PH, 128, PB, 2, 37]
        pad = np.zeros((BC, 256, 37), dtype=np.float32)
        pad[:, 0:T] = ub_c
        natb = np.ascontiguousarray(
            pad.reshape(PH, PB, 2, 128, 37).transpose(0, 3, 1, 2, 4)
        ).reshape(PH, 128, PB * 2 * 37).astype(BF_NP)
        waug_r = waug[sl].reshape(PH, 2, PB // 2, 37, 80).transpose(0, 3, 1, 2, 4)
        waugt = np.zeros((PH, 128, PB // 2, 80), dtype=np.float32)
        waugt[:, 0:37] = waug_r[:, :, 0]
        waugt[:, 64:101] = waug_r[:, :, 1]
        waugt = waugt.reshape(PH, 128, PB * 40).astype(BF_NP)
        lensf = lens[sl].astype(np.float32)
        lensc = np.zeros((104, 2 * PH), dtype=np.float32)
        for p in range(104):
            if p % 32 < 8:
                for ph in range(PH):
                    for hc in range(2):
                        bb = 64 * ph + 16 * (p // 32) + 2 * (p % 32) + hc
                        lensc[p, 2 * ph + hc] = lensf[bb]
        in_maps.append({
            "ubt": ubt, "natb": natb, "waugt": waugt,
            "w2p": w2p, "w3p": w3p, "b2c": b2c, "b3c": b3c,
            "lens": lensc,
        })
    return in_maps


_NC_CACHE = {}


def get_module():
    if "nc" not in _NC_CACHE:
        _NC_CACHE["nc"] = build_module()
    return _NC_CACHE["nc"]


def kernel(query_ad, user_behavior, user_behavior_length,
           W1, b1, W2, b2, W3, b3, trace=False):
    nc = get_module()
    in_maps = host_prep(query_ad, user_behavior, user_behavior_length,
                        W1, b1, W2, b2, W3, b3)
    res = run_bass_kernel_spmd(nc, in_maps, core_ids=list(range(N_CORES)),
                               trace=trace)
    outs = [np.asarray(res.results[c]["out"])[:, 0:36] for c in range(N_CORES)]
    full = np.ascontiguousarray(np.concatenate(outs, axis=0)).reshape(B, 1, 36)
    kernel.last_results = res.results
    if trace:
        kernel.last_result = res
    return full
